# revision 40
# baseline (speedup 1.0000x reference)
"""GQA attention (B=2,S=1024,HID=2048,NH=32,NKV=8,HD=64) on 8 TRN2 cores.

Sharding: core c -> batch b=c//4, head-group g=c%4 (8 q heads / 2 kv heads).
Core computes partial out[b] = attn(heads of g) @ Wo[rows of g]; host sums the
4 row-parallel partials per batch.

Head pairing: local q heads are reordered [0,4,1,5,2,6,3,7] so q chunk mc
holds (kv0 head mc) on partitions 0:64 and (kv1 head mc) on 64:128. K proj
then emits both kv heads in ONE [128,S] chunk (kv0 rows 0:64, kv1 rows
64:128) with no replication, and scores use krep[r:r+64] with r=64*(h%2).

Device dataflow (matmuls bf16 -> fp32 PSUM), software-pipelined so the PE
never sits on an ACT/DVE result (PE p-state ramps only under continuous
execution):
  K proj -> V proj (covers ACT copy of K) -> rot(K) -> Q0 -> V transposes
  -> Q1 -> rot(Q0) -> Q2 -> rot(Q1) -> Q3 -> rot(Q2) -> rot(Q3)
  -> per head h: sc(0); for kc: [sc(kc+1); exp(kc) on ACT; PV(kc)]
     even heads get the softmax denominator fused via augmented [V|1] lhsT
     (PSUM row 64); odd heads use a ones-column matmul into row 0
  -> normalize: DVE reciprocal -> gpsimd partition_broadcast -> DVE mult
  -> out proj (stationary Wo) -> DMA out [2048,1024] f32 partials
"""

import numpy as np
import ml_dtypes

import concourse.bass as bass
import concourse.bacc as bacc
import concourse.mybir as mybir
from concourse.tile import TileContext
from concourse.bass_utils import run_bass_kernel_spmd
from concourse.masks import make_identity

B, S, HID = 2, 1024, 2048
NH, NKV, HD = 32, 8, 64
G = 4                      # head groups (tensor-parallel degree per batch)
QH = NH // G               # 8 q heads per core
KVH = NKV // G             # 2 kv heads per core
QD = QH * HD               # 512
ROPE_BASE = 10000.0
BF16 = mybir.dt.bfloat16
F32 = mybir.dt.float32
NEG_BIG = float(np.finfo(np.float32).min)

KC = S // 128              # 8 k-token chunks
HC = HID // 128            # 16 hidden chunks
QC = QD // 128             # 4 q-dim chunks (head pairs)

LAST_RESULT = None
_CACHE = {}


def _build(use_mask: bool) -> bass.Bass:
    nc = bacc.Bacc(None, target_bir_lowering=False)
    # weights arrive pre-packed on host into the SBUF tile layout so every
    # DMA is a contiguous 128-partition copy (few descriptors, fast issue)
    hsT_d = nc.dram_tensor("hsT", [HID, S], BF16, kind="ExternalInput")
    wq_d = nc.dram_tensor("wq", [128, QC * HC * 128], BF16,
                          kind="ExternalInput")
    wk_d = nc.dram_tensor("wk", [128, HC * KVH * HD], BF16,
                          kind="ExternalInput")
    wv_d = nc.dram_tensor("wv", [128, HC * KVH * HD], BF16,
                          kind="ExternalInput")
    wo_d = nc.dram_tensor("wo", [128, QC * HID], BF16, kind="ExternalInput")
    cos_d = nc.dram_tensor("cos2", [128, S], BF16, kind="ExternalInput")
    sin_d = nc.dram_tensor("sin2", [128, S], BF16, kind="ExternalInput")
    perm_d = nc.dram_tensor("permT", [128, 128], BF16, kind="ExternalInput")
    if use_mask:
        mask_d = nc.dram_tensor("maskT", [S, S], BF16, kind="ExternalInput")
    out_d = nc.dram_tensor("out", [HID, S], BF16, kind="ExternalOutput")

    with TileContext(nc) as tc:
        with (
            tc.tile_pool(name="resid", bufs=1) as rp,
            tc.tile_pool(name="work", bufs=2) as wp,
            tc.tile_pool(name="exps", bufs=6) as ep,
            tc.tile_pool(name="outs", bufs=3) as op_,
        ):
            # ---- input DMAs: split across the two HWDGE queues (sync +
            # scalar) so descriptor issue (~0.6us each) doesn't serialize.
            # Order: K/V weights first, then hsT chunks (K/V proj chase
            # them), then wq block 0, rope tables, wq 1-3, wo last ----
            # Arrival order matches consumption order: each queue leads with
            # the weight its first projection needs (wk / wq block 0), then
            # its half of the hsT chunks (K+Q0 proj chase them chunk-wise),
            # then later-needed tensors.
            # wk/wq0 split into a 4-chunk head + tail: the first projection
            # matmuls gate on a 128KB transfer instead of 512KB (early DMA
            # bandwidth is contended across all 8 cores)
            wk = rp.tile([128, HC * KVH * HD], BF16, tag="wk")
            nc.sync.dma_start(out=wk[:, 0:512], in_=wk_d[:, 0:512])
            wqb = rp.tile([128, QC * HC * 128], BF16, tag="wqb")
            nc.scalar.dma_start(out=wqb[:, 0:512], in_=wq_d[:, 0:512])
            hsT = [None] * HC
            for k in range(HC):
                t = rp.tile([128, S], BF16, tag=f"hsT{k}", name=f"hsT{k}")
                eng = nc.sync if k % 2 == 0 else nc.scalar
                if k < 2:
                    # first chunks in halves: the opening matmul gates on a
                    # 128KB transfer instead of 256KB
                    eng.dma_start(out=t[:, 0:512],
                                  in_=hsT_d[k * 128:(k + 1) * 128, 0:512])
                    eng.dma_start(out=t[:, 512:S],
                                  in_=hsT_d[k * 128:(k + 1) * 128, 512:S])
                else:
                    eng.dma_start(out=t[:],
                                  in_=hsT_d[k * 128:(k + 1) * 128, :])
                hsT[k] = t
                if k == 1:
                    nc.sync.dma_start(out=wk[:, 512:1024],
                                      in_=wk_d[:, 512:1024])
                    nc.scalar.dma_start(out=wqb[:, 512:1024],
                                        in_=wq_d[:, 512:1024])
                if k == 3:
                    nc.sync.dma_start(
                        out=wk[:, 1024:HC * KVH * HD],
                        in_=wk_d[:, 1024:HC * KVH * HD],
                    )
                    nc.scalar.dma_start(
                        out=wqb[:, 1024:HC * 128], in_=wq_d[:, 1024:HC * 128]
                    )
            permT = rp.tile([128, 128], BF16, tag="permT")
            nc.scalar.dma_start(out=permT[:], in_=perm_d[:, :])
            wv = rp.tile([128, HC * KVH * HD], BF16, tag="wv")
            nc.scalar.dma_start(out=wv[:], in_=wv_d[:, :])
            cos2 = rp.tile([128, S], BF16, tag="cos2")
            nc.scalar.dma_start(out=cos2[:], in_=cos_d[:, :])
            sin2 = rp.tile([128, S], BF16, tag="sin2")
            nc.scalar.dma_start(out=sin2[:], in_=sin_d[:, :])
            for mc in range(1, QC):
                nc.sync.dma_start(
                    out=wqb[:, mc * HC * 128:(mc + 1) * HC * 128],
                    in_=wq_d[:, mc * HC * 128:(mc + 1) * HC * 128],
                )
            wo = rp.tile([128, QC * HID], BF16, tag="wo")
            nc.sync.dma_start(out=wo[:], in_=wo_d[:, :])
            if use_mask:
                maskT = rp.tile([128, KC * S], BF16, tag="maskT")
                nc.sync.dma_start(
                    out=maskT[:].rearrange("p (k q) -> p k q", k=KC),
                    in_=mask_d[:, :].rearrange("(k p) q -> p k q", p=128),
                )
            # ---- persistent intermediates ----
            qrot = rp.tile([128, QC * S], BF16, tag="qrot")
            # krep variants zero-padded to full 128 contraction rows: even
            # heads use [K_kv0; 0], odd use [0; K_kv1] so every scores matmul
            # runs the full 128x128 PE array (zeros kill the other head's q).
            krepE = rp.tile([128, S], BF16, tag="krepE")
            nc.any.memset(krepE[64:128, :], 0.0)
            krepO = rp.tile([128, S], BF16, tag="krepO")
            nc.any.memset(krepO[0:64, :], 0.0)
            # PV lhsT tiles, full 128 cols per kc chunk so every PV matmul
            # runs the whole PE array with the softmax denominator fused:
            #   vaugE = [V_kv0 | ones]: PV rows 0:64, denominator on row 64+
            #   vaugO = [ones | V_kv1]: denominator on rows 0:63, PV 64:128
            vaugE = rp.tile([128, KC * 128], BF16, tag="vaugE")
            nc.any.memset(vaugE[:], 1.0)
            vaugO = rp.tile([128, KC * 128], BF16, tag="vaugO")
            nc.any.memset(vaugO[:], 1.0)
            attnT = rp.tile([128, QC * S], BF16, tag="attnT")
            ident = rp.tile([128, 128], BF16, tag="ident")
            make_identity(nc, ident[:])
            # SBUF staging for the last pair so the attention PSUM pools can
            # close right after the final PV (normalize finishes during the
            # out-projection instead of serializing before it)
            lastU = {6: rp.tile([128, S], F32, tag="lastU6", name="lastU6"),
                     7: rp.tile([128, S], F32, tag="lastU7", name="lastU7")}

            # ================= projections + rope =================
            # pj: 3 bufs x [128,S] f32 (12KB/part) + tp: 2 x [128,S] bf16
            # (4KB) = 16KB PSUM. Each tp tile owns a full 2KB zero region so
            # the V transposes don't invalidate each other.
            with (
                tc.tile_pool(name="pj", bufs=3, space="PSUM") as pj,
                tc.tile_pool(name="tp", bufs=2, space="PSUM") as tpp,
            ):
                def proj(w_ap_fn, tag):
                    ps = pj.tile([128, S], F32, tag="pj", name=f"ps_{tag}")
                    for ns in range(2):
                        for k in range(HC):
                            nc.tensor.matmul(
                                ps[:, ns * 512:(ns + 1) * 512],
                                w_ap_fn(k),
                                hsT[k][:, ns * 512: ns * 512 + 512],
                                start=(k == 0), stop=(k == HC - 1),
                            )
                    return ps

                def proj2(wa_fn, wb_fn, tag):
                    """Two projections interleaved per hsT chunk so the PE
                    keeps pace with the chunk-wise hsT DMA arrival."""
                    pa = pj.tile([128, S], F32, tag="pj", name=f"ps_{tag}a")
                    pb = pj.tile([128, S], F32, tag="pj", name=f"ps_{tag}b")
                    for k in range(HC):
                        for ps, w_fn in ((pa, wa_fn), (pb, wb_fn)):
                            for ns in range(2):
                                nc.tensor.matmul(
                                    ps[:, ns * 512:(ns + 1) * 512],
                                    w_fn(k),
                                    hsT[k][:, ns * 512: ns * 512 + 512],
                                    start=(k == 0), stop=(k == HC - 1),
                                )
                    return pa, pb

                def rot_cast(ps, tag):
                    # ACT cast to bf16 per 512-col half: subtile deps let the
                    # first half start while the projection's ns=1 matmuls
                    # still run, and ACT is idle pre-attention anyway
                    raw = wp.tile([128, S], BF16, tag="raw", name=f"raw_{tag}")
                    for ns in range(2):
                        sl = slice(ns * 512, (ns + 1) * 512)
                        nc.scalar.activation(
                            raw[:, sl], ps[:, sl],
                            mybir.ActivationFunctionType.Copy
                        )
                    return raw

                def rot_apply(raw, dsts, tag, pool=None, pool_tag="pj"):
                    """PE rotate-half matmul + DVE combine with cos/sin, per
                    512-col half to shorten the dependency chain. dsts:
                    (dst_tile, row_lo, row_hi, col_base)."""
                    pl = pj if pool is None else pool
                    ps_rot = pl.tile([128, S], F32, tag=pool_tag,
                                     name=f"rot_{tag}")
                    t1 = wp.tile([128, S], BF16, tag="t1", name=f"t1_{tag}")
                    t2 = wp.tile([128, S], BF16, tag="t2", name=f"t2_{tag}")
                    for ns in range(2):
                        sl = slice(ns * 512, (ns + 1) * 512)
                        nc.tensor.matmul(
                            ps_rot[:, sl], permT[:], raw[:, sl],
                            start=True, stop=True,
                        )
                        nc.vector.tensor_tensor(
                            t1[:, sl], raw[:, sl], cos2[:, sl],
                            mybir.AluOpType.mult
                        )
                        nc.vector.tensor_tensor(
                            t2[:, sl], ps_rot[:, sl], sin2[:, sl],
                            mybir.AluOpType.mult
                        )
                        for dst, lo, hi, col0 in dsts:
                            nc.vector.tensor_tensor(
                                dst[lo:hi, col0 + ns * 512:col0 + ns * 512
                                    + 512],
                                t1[lo:hi, sl], t2[lo:hi, sl],
                                mybir.AluOpType.add,
                            )

                def wq_ap(mc):
                    return lambda k: wqb[:, mc * HC * 128 + k * 128:
                                         mc * HC * 128 + (k + 1) * 128]

                def qdst(mc):
                    return [(qrot, 0, 128, mc * S)]

                # K and Q0 projections interleaved per hsT chunk (chase the
                # chunk-wise hsT DMA arrival); V proj next covers the rot
                # cast/combine latency of K and Q0; each later Q rot lags one
                # projection so its chain hides under the next proj's MMs
                ps_q = [None] * QC
                raw_q = [None] * QC
                ps_k, ps_q[0] = proj2(
                    lambda k: wk[:, k * 128:(k + 1) * 128],
                    wq_ap(0),
                    "kq0",
                )
                raw_k = rot_cast(ps_k, "k")
                raw_q[0] = rot_cast(ps_q[0], "q0")
                # V proj as V^T (stationary wv)
                ps_vt = proj(lambda k: wv[:, k * 128:(k + 1) * 128], "v")
                rot_apply(raw_k, [
                    (krepE, 0, 64, 0),
                    (krepO, 64, 128, 0),
                ], "k")
                rot_apply(raw_q[0], qdst(0), "q0")
                vt_sb = wp.tile([128, S], BF16, tag="vts")
                nc.scalar.activation(
                    vt_sb[:], ps_vt[:], mybir.ActivationFunctionType.Copy
                )
                ps_q[1] = proj(wq_ap(1), "q1")
                raw_q[1] = rot_cast(ps_q[1], "q1")
                # PE-transpose each token chunk into vaugE/vaugO; each
                # transpose gets its own PSUM bank (2-tile rotation)
                for t in range(KC):
                    ps_tr = tpp.tile([128, S], BF16, tag="tp", name=f"tr{t}")
                    nc.tensor.transpose(
                        ps_tr[:, 0:128], vt_sb[:, t * 128:(t + 1) * 128],
                        ident[:]
                    )
                    nc.vector.tensor_copy(
                        vaugE[:, t * 128:t * 128 + 64], ps_tr[:, 0:64]
                    )
                    nc.vector.tensor_copy(
                        vaugO[:, t * 128 + 64:(t + 1) * 128], ps_tr[:, 64:128]
                    )
                for mc in range(2, QC):
                    ps_q[mc] = proj(wq_ap(mc), f"q{mc}")
                    raw_q[mc] = rot_cast(ps_q[mc], f"q{mc}")
                    if mc < QC - 1:
                        rot_apply(raw_q[mc - 1], qdst(mc - 1), f"q{mc-1}")
                # rots of the last two Q pairs are emitted inside the
                # attention block (ps_rot from the st pool) so their DVE
                # chains hide under the first head's scores/exp stream —
                # qrot chunks 2/3 are only needed from head 4 onward

            # ================= attention =================
            # PSUM: st 2 x [128,S] f32 (8KB) + av 2 x [128,S] f32 (8KB) = 16KB
            def normalize(h, dsrc, psrc, staged=False):
                """Denominator row dr of dsrc -> reciprocal -> broadcast ->
                scale psrc's PV rows into attnT. staged=True when dsrc/psrc
                already live in SBUF."""
                mc, par = h // 2, h % 2
                r = par * 64
                dr = 64 if par == 0 else 0
                recip = wp.tile([128, S], F32, tag="recip", name=f"recip{h}")
                # ~18-bit approx reciprocal: single DVE op (the exact
                # `reciprocal` is 8 cyc/elem on ONE lane here -> 6.5us).
                # The custom op only works at base partition 0 (PSUM reads
                # are fine), so even heads hop the denominator row to
                # partition 0 FIRST (DVE stage to SBUF since DMA can't read
                # PSUM, then DMA hop), then recip at partition 0.
                if dr != 0:
                    if not staged:     # denominator still in PSUM
                        den = wp.tile([128, S], F32, tag="den",
                                      name=f"den{h}")
                        nc.vector.tensor_copy(
                            den[dr:dr + 1, :], dsrc[dr:dr + 1, :]
                        )
                        hop_src = den
                    else:
                        hop_src = dsrc
                    den0 = wp.tile([1, S], F32, tag="den0", name=f"den0{h}")
                    nc.sync.dma_start(
                        out=den0[0:1, :], in_=hop_src[dr:dr + 1, :]
                    )
                    din = den0[0:1, :]
                else:
                    din = dsrc[0:1, :]
                nc.vector.reciprocal_approx_fast(recip[0:1, :], din)
                bc = wp.tile([128, S], F32, tag="bcast", name=f"bc{h}")
                nc.gpsimd.partition_broadcast(bc[:], recip[0:1, :])
                nc.vector.tensor_tensor(
                    attnT[r:r + 64, mc * S:(mc + 1) * S],
                    psrc[r:r + 64, :], bc[r:r + 64, :],
                    mybir.AluOpType.mult,
                )

            def op_mm(pso, mc2, ns, kc2):
                nc.tensor.matmul(
                    pso[:, ns * 512:(ns + 1) * 512],
                    wo[:, kc2 * HID + mc2 * 128:
                       kc2 * HID + (mc2 + 1) * 128],
                    attnT[:, kc2 * S + ns * 512:
                          kc2 * S + ns * 512 + 512],
                    start=(kc2 == 0), stop=(kc2 == QC - 1),
                )

            def op_fin(pso, mc2):
                # bf16 partials: host sums the 4 row-parallel partials in
                # f32; halves the output DMA (8MB -> 4MB per core).
                # PSUM->SBUF copies alternate between DVE and ACT and the
                # output-DMA issue between the two HWDGE queues.
                osb = op_.tile([128, S], BF16, tag="osb")
                if mc2 % 2 == 0:
                    nc.vector.tensor_copy(osb[:], pso[:])
                    nc.sync.dma_start(
                        out=out_d[mc2 * 128:(mc2 + 1) * 128, :], in_=osb[:]
                    )
                else:
                    nc.scalar.activation(
                        osb[:], pso[:], mybir.ActivationFunctionType.Copy
                    )
                    nc.scalar.dma_start(
                        out=out_d[mc2 * 128:(mc2 + 1) * 128, :], in_=osb[:]
                    )

            with (
                tc.tile_pool(name="st", bufs=2, space="PSUM") as stp,
                tc.tile_pool(name="av", bufs=2, space="PSUM") as avp,
            ):
                # last two Q rots draw their PSUM from the av pool: av_h0's
                # first PV comes ~4us into attention, so the rot chains are
                # fully hidden and never stall the scores/exp rotation
                rot_apply(raw_q[QC - 2], qdst(QC - 2), f"q{QC-2}",
                          pool=avp, pool_tag="av")
                rot_apply(raw_q[QC - 1], qdst(QC - 1), f"q{QC-1}",
                          pool=avp, pool_tag="av")
                for h in range(QH):
                    mc = h // 2                  # q chunk / pair
                    par = h % 2                  # kv head = parity
                    r = par * 64                 # partition row base
                    krep = krepE if par == 0 else krepO

                    def scores(kc):
                        ps_st = stp.tile([128, S], F32, tag="st",
                                         name=f"st_h{h}k{kc}")
                        for ns in range(2):
                            nc.tensor.matmul(
                                ps_st[:, ns * 512:(ns + 1) * 512],
                                krep[:, kc * 128:(kc + 1) * 128],
                                qrot[:,
                                     mc * S + ns * 512: mc * S + ns * 512 + 512],
                                start=True, stop=True,
                            )
                        return ps_st

                    ps_at = avp.tile([128, S], F32, tag="av", name=f"av_h{h}")
                    st_tiles = {0: scores(0)}
                    for kc in range(KC):
                        if kc + 1 < KC:
                            st_tiles[kc + 1] = scores(kc + 1)
                        ps_st = st_tiles.pop(kc)
                        if use_mask:
                            nc.vector.tensor_tensor(
                                ps_st[:], ps_st[:],
                                maskT[:, kc * S:(kc + 1) * S],
                                mybir.AluOpType.add,
                            )
                        ex = ep.tile([128, S], BF16, tag="ex",
                                     name=f"ex_h{h}k{kc}")
                        nc.scalar.activation(
                            ex[:], ps_st[:], mybir.ActivationFunctionType.Exp
                        )
                        va = vaugE if par == 0 else vaugO
                        for ns in range(2):
                            nc.tensor.matmul(
                                ps_at[:, ns * 512:(ns + 1) * 512],
                                va[:, kc * 128:(kc + 1) * 128],
                                ex[:, ns * 512:(ns + 1) * 512],
                                start=(kc == 0), stop=(kc == KC - 1),
                            )
                    # normalize; denominator on row 64 (even) / row 0 (odd);
                    # even heads hop the reciprocal row to partition 0 for
                    # the gpsimd broadcast (HW broadcast reads partition 0)
                    if h >= QH - 2:
                        # last pair: ONE full-tile copy stages PV rows AND
                        # the denominator row to SBUF so the PSUM pools can
                        # close right after the final PV
                        nc.vector.tensor_copy(lastU[h][:], ps_at[:])
                        normalize(h, lastU[h], lastU[h], staged=True)
                        continue
                    normalize(h, ps_at, ps_at)

                # ---- early out-projection: chunks 0-3 accumulate in the
                # av/st slots freed by the last pair's SBUF staging, pulling
                # ~10us of out-proj PE work under the attention tail ----
                early = []
                for i, (pl, tg) in enumerate(
                        [(avp, "av"), (avp, "av"), (stp, "st"), (stp, "st")]):
                    pso = pl.tile([128, S], F32, tag=tg, name=f"wop_e{i}")
                    early.append(pso)
                    for ns in range(2):
                        for kc2 in range(QC - 1):
                            op_mm(pso, i, ns, kc2)
                for i, pso in enumerate(early):
                    for ns in range(2):
                        op_mm(pso, i, ns, QC - 1)
                    op_fin(pso, i)

            # ================= output projection (transposed out) ==========
            with tc.tile_pool(name="wop", bufs=4, space="PSUM") as wop:
                for mc2 in range(4, HID // 128):
                    pso = wop.tile([128, S], F32, tag="wop")
                    for ns in range(2):
                        for kc2 in range(QC):
                            op_mm(pso, mc2, ns, kc2)
                    op_fin(pso, mc2)
    nc.finalize()
    return nc


def _rope_tables():
    inv = 1.0 / (ROPE_BASE ** (np.arange(0, HD, 2, dtype=np.float32) / HD))
    t = np.arange(S, dtype=np.float32)
    freqs = np.outer(t, inv)
    emb = np.concatenate([freqs, freqs], axis=-1)  # [S, HD]
    return np.cos(emb).astype(np.float32), np.sin(emb).astype(np.float32)


def _perm_T():
    P = np.zeros((128, 128), dtype=np.float32)
    for blk in range(2):
        o = blk * 64
        for i in range(32):
            P[o + i, o + i + 32] = -1.0
            P[o + i + 32, o + i] = 1.0
    return P.T.astype(ml_dtypes.bfloat16)


# local head order: pair mc = (kv0 head mc, kv1 head mc)
_HEAD_PERM = [0, 4, 1, 5, 2, 6, 3, 7]


def _head_cols(g):
    cols = []
    for lh in _HEAD_PERM:
        s0 = (g * QH + lh) * HD
        cols.append(np.arange(s0, s0 + HD))
    return np.concatenate(cols)


def _core_weights(g, Wq, Wk, Wv, Wo, scale):
    """Pack a core's weight slices into the SBUF tile layouts
    (partition-major, hid-chunked) so each DMA is contiguous."""
    bf = ml_dtypes.bfloat16
    cols = _head_cols(g)
    wq_c = (Wq[:, cols] * scale).astype(bf)          # [2048, 512]
    wq_pack = np.ascontiguousarray(                  # mc-major
        wq_c.reshape(HC, 128, QC, 128).transpose(1, 2, 0, 3)
        .reshape(128, QC * HC * 128))
    wk_c = Wk[:, g * KVH * HD:(g + 1) * KVH * HD].astype(bf)
    wk_pack = np.ascontiguousarray(
        wk_c.reshape(HC, 128, KVH * HD).transpose(1, 0, 2)
        .reshape(128, HC * KVH * HD))
    wv_c = Wv[:, g * KVH * HD:(g + 1) * KVH * HD].astype(bf)
    wv_pack = np.ascontiguousarray(
        wv_c.reshape(HC, 128, KVH * HD).transpose(1, 0, 2)
        .reshape(128, HC * KVH * HD))
    wo_c = Wo[cols, :].astype(bf)                    # [512, 2048]
    wo_pack = np.ascontiguousarray(
        wo_c.reshape(QC, 128, HID).transpose(1, 0, 2)
        .reshape(128, QC * HID))
    return {"wq": wq_pack, "wk": wk_pack, "wv": wv_pack, "wo": wo_pack}


def kernel(hidden_states, position_ids, attention_mask, Wq, Wk, Wv, Wo,
           _trace=False):
    global LAST_RESULT
    bf = ml_dtypes.bfloat16
    hidden_states = np.asarray(hidden_states, dtype=np.float32)
    Wq = np.asarray(Wq, dtype=np.float32)
    Wk = np.asarray(Wk, dtype=np.float32)
    Wv = np.asarray(Wv, dtype=np.float32)
    Wo = np.asarray(Wo, dtype=np.float32)
    mask = np.asarray(attention_mask, dtype=np.float32)
    pos = np.asarray(position_ids).astype(np.int64)

    use_mask = bool(np.any(mask))
    key = use_mask
    if key not in _CACHE:
        _CACHE[key] = _build(use_mask)
    nc = _CACHE[key]

    cos_t, sin_t = _rope_tables()
    permT = _perm_T()
    scale = 1.0 / np.sqrt(HD)

    hsT_b = [np.ascontiguousarray(hidden_states[b].T).astype(bf)
             for b in range(B)]
    cos2_b, sin2_b = [], []
    for b in range(B):
        cos2_b.append(np.ascontiguousarray(
            np.tile(cos_t[pos[b]].T, (2, 1))).astype(bf))
        sin2_b.append(np.ascontiguousarray(
            np.tile(sin_t[pos[b]].T, (2, 1))).astype(bf))
    if use_mask:
        maskT_full = np.ascontiguousarray(
            np.maximum(mask[:, 0], NEG_BIG).transpose(0, 2, 1)).astype(bf)

    in_maps = []
    for c in range(8):
        b, g = c // G, c % G
        m = _core_weights(g, Wq, Wk, Wv, Wo, scale)
        m.update({
            "hsT": hsT_b[b],
            "permT": permT,
            "cos2": cos2_b[b],
            "sin2": sin2_b[b],
        })
        if use_mask:
            m["maskT"] = maskT_full[b]
        in_maps.append(m)

    res = run_bass_kernel_spmd(nc, in_maps, core_ids=list(range(8)),
                               trace=_trace)
    LAST_RESULT = res
    out = np.zeros((B, S, HID), dtype=np.float32)
    for c in range(8):
        out[c // G] += res.results[c]["out"].astype(np.float32).T
    return out



# revision 42
# speedup vs baseline: 1.0305x; 1.0305x over previous
"""GQA attention (B=2,S=1024,HID=2048,NH=32,NKV=8,HD=64) on 8 TRN2 cores.

Sharding: core c -> batch b=c//4, head-group g=c%4 (8 q heads / 2 kv heads).
Core computes partial out[b] = attn(heads of g) @ Wo[rows of g]; host sums the
4 row-parallel partials per batch.

Head pairing: local q heads are reordered [0,4,1,5,2,6,3,7] so q chunk mc
holds (kv0 head mc) on partitions 0:64 and (kv1 head mc) on 64:128. K proj
then emits both kv heads in ONE [128,S] chunk (kv0 rows 0:64, kv1 rows
64:128) with no replication, and scores use krep[r:r+64] with r=64*(h%2).

Device dataflow (matmuls bf16 -> fp32 PSUM), software-pipelined so the PE
never sits on an ACT/DVE result (PE p-state ramps only under continuous
execution):
  K proj -> V proj (covers ACT copy of K) -> rot(K) -> Q0 -> V transposes
  -> Q1 -> rot(Q0) -> Q2 -> rot(Q1) -> Q3 -> rot(Q2) -> rot(Q3)
  -> per head h: sc(0); for kc: [sc(kc+1); exp(kc) on ACT; PV(kc)]
     even heads get the softmax denominator fused via augmented [V|1] lhsT
     (PSUM row 64); odd heads use a ones-column matmul into row 0
  -> normalize: DVE reciprocal -> gpsimd partition_broadcast -> DVE mult
  -> out proj (stationary Wo) -> DMA out [2048,1024] f32 partials
"""

import numpy as np
import ml_dtypes

import concourse.bass as bass
import concourse.bacc as bacc
import concourse.mybir as mybir
from concourse.tile import TileContext
from concourse.bass_utils import run_bass_kernel_spmd
from concourse.masks import make_identity

B, S, HID = 2, 1024, 2048
NH, NKV, HD = 32, 8, 64
G = 4                      # head groups (tensor-parallel degree per batch)
QH = NH // G               # 8 q heads per core
KVH = NKV // G             # 2 kv heads per core
QD = QH * HD               # 512
ROPE_BASE = 10000.0
BF16 = mybir.dt.bfloat16
F32 = mybir.dt.float32
NEG_BIG = float(np.finfo(np.float32).min)

KC = S // 128              # 8 k-token chunks
HC = HID // 128            # 16 hidden chunks
QC = QD // 128             # 4 q-dim chunks (head pairs)

LAST_RESULT = None
_CACHE = {}


def _build(use_mask: bool) -> bass.Bass:
    nc = bacc.Bacc(None, target_bir_lowering=False)
    # weights arrive pre-packed on host into the SBUF tile layout so every
    # DMA is a contiguous 128-partition copy (few descriptors, fast issue)
    hsT_d = nc.dram_tensor("hsT", [HID, S], BF16, kind="ExternalInput")
    wq_d = nc.dram_tensor("wq", [128, QC * HC * 128], BF16,
                          kind="ExternalInput")
    wk_d = nc.dram_tensor("wk", [128, HC * KVH * HD], BF16,
                          kind="ExternalInput")
    wv_d = nc.dram_tensor("wv", [128, HC * KVH * HD], BF16,
                          kind="ExternalInput")
    wo_d = nc.dram_tensor("wo", [128, QC * HID], BF16, kind="ExternalInput")
    cos_d = nc.dram_tensor("cos2", [128, S], BF16, kind="ExternalInput")
    sin_d = nc.dram_tensor("sin2", [128, S], BF16, kind="ExternalInput")
    perm_d = nc.dram_tensor("permT", [128, 128], BF16, kind="ExternalInput")
    if use_mask:
        mask_d = nc.dram_tensor("maskT", [S, S], BF16, kind="ExternalInput")
    out_d = nc.dram_tensor("out", [HID, S], BF16, kind="ExternalOutput")

    with TileContext(nc) as tc:
        with (
            tc.tile_pool(name="resid", bufs=1) as rp,
            tc.tile_pool(name="work", bufs=2) as wp,
            tc.tile_pool(name="exps", bufs=6) as ep,
            tc.tile_pool(name="outs", bufs=3) as op_,
        ):
            # ---- input DMAs: split across the two HWDGE queues (sync +
            # scalar) so descriptor issue (~0.6us each) doesn't serialize.
            # Order: K/V weights first, then hsT chunks (K/V proj chase
            # them), then wq block 0, rope tables, wq 1-3, wo last ----
            # Arrival order matches consumption order: each queue leads with
            # the weight its first projection needs (wk / wq block 0), then
            # its half of the hsT chunks (K+Q0 proj chase them chunk-wise),
            # then later-needed tensors.
            # wk/wq0 split into a 4-chunk head + tail: the first projection
            # matmuls gate on a 128KB transfer instead of 512KB (early DMA
            # bandwidth is contended across all 8 cores)
            wk = rp.tile([128, HC * KVH * HD], BF16, tag="wk")
            nc.sync.dma_start(out=wk[:, 0:512], in_=wk_d[:, 0:512])
            wqb = rp.tile([128, QC * HC * 128], BF16, tag="wqb")
            nc.scalar.dma_start(out=wqb[:, 0:512], in_=wq_d[:, 0:512])
            hsT = [None] * HC
            for k in range(HC):
                t = rp.tile([128, S], BF16, tag=f"hsT{k}", name=f"hsT{k}")
                eng = nc.sync if k % 2 == 0 else nc.scalar
                if k < 2:
                    # first chunks in halves: the opening matmuls gate on a
                    # 128KB transfer instead of 256KB
                    eng.dma_start(out=t[:, 0:512],
                                  in_=hsT_d[k * 128:(k + 1) * 128, 0:512])
                    eng.dma_start(out=t[:, 512:S],
                                  in_=hsT_d[k * 128:(k + 1) * 128, 512:S])
                else:
                    eng.dma_start(out=t[:],
                                  in_=hsT_d[k * 128:(k + 1) * 128, :])
                hsT[k] = t
                if k == 1:
                    nc.sync.dma_start(out=wk[:, 512:1024],
                                      in_=wk_d[:, 512:1024])
                    nc.scalar.dma_start(out=wqb[:, 512:1024],
                                        in_=wq_d[:, 512:1024])
                if k == 3:
                    nc.sync.dma_start(
                        out=wk[:, 1024:HC * KVH * HD],
                        in_=wk_d[:, 1024:HC * KVH * HD],
                    )
                    nc.scalar.dma_start(
                        out=wqb[:, 1024:HC * 128], in_=wq_d[:, 1024:HC * 128]
                    )
            permT = rp.tile([128, 128], BF16, tag="permT")
            nc.scalar.dma_start(out=permT[:], in_=perm_d[:, :])
            wv = rp.tile([128, HC * KVH * HD], BF16, tag="wv")
            nc.scalar.dma_start(out=wv[:], in_=wv_d[:, :])
            cos2 = rp.tile([128, S], BF16, tag="cos2")
            nc.scalar.dma_start(out=cos2[:], in_=cos_d[:, :])
            sin2 = rp.tile([128, S], BF16, tag="sin2")
            nc.scalar.dma_start(out=sin2[:], in_=sin_d[:, :])
            for mc in range(1, QC):
                nc.sync.dma_start(
                    out=wqb[:, mc * HC * 128:(mc + 1) * HC * 128],
                    in_=wq_d[:, mc * HC * 128:(mc + 1) * HC * 128],
                )
            wo = rp.tile([128, QC * HID], BF16, tag="wo")
            nc.sync.dma_start(out=wo[:], in_=wo_d[:, :])
            if use_mask:
                maskT = rp.tile([128, KC * S], BF16, tag="maskT")
                nc.sync.dma_start(
                    out=maskT[:].rearrange("p (k q) -> p k q", k=KC),
                    in_=mask_d[:, :].rearrange("(k p) q -> p k q", p=128),
                )
            # ---- persistent intermediates ----
            qrot = rp.tile([128, QC * S], BF16, tag="qrot")
            # krep variants zero-padded to full 128 contraction rows: even
            # heads use [K_kv0; 0], odd use [0; K_kv1] so every scores matmul
            # runs the full 128x128 PE array (zeros kill the other head's q).
            krepE = rp.tile([128, S], BF16, tag="krepE")
            nc.any.memset(krepE[64:128, :], 0.0)
            krepO = rp.tile([128, S], BF16, tag="krepO")
            nc.any.memset(krepO[0:64, :], 0.0)
            # PV lhsT tiles, full 128 cols per kc chunk so every PV matmul
            # runs the whole PE array with the softmax denominator fused:
            #   vaugE = [V_kv0 | ones]: PV rows 0:64, denominator on row 64+
            #   vaugO = [ones | V_kv1]: denominator on rows 0:63, PV 64:128
            vaugE = rp.tile([128, KC * 128], BF16, tag="vaugE")
            nc.any.memset(vaugE[:], 1.0)
            vaugO = rp.tile([128, KC * 128], BF16, tag="vaugO")
            nc.any.memset(vaugO[:], 1.0)
            attnT = rp.tile([128, QC * S], BF16, tag="attnT")
            ident = rp.tile([128, 128], BF16, tag="ident")
            make_identity(nc, ident[:])
            # SBUF staging for the last pair so the attention PSUM pools can
            # close right after the final PV (normalize finishes during the
            # out-projection instead of serializing before it)
            lastU = {6: rp.tile([128, S], F32, tag="lastU6", name="lastU6"),
                     7: rp.tile([128, S], F32, tag="lastU7", name="lastU7")}

            # ================= projections + rope =================
            # pj: 3 bufs x [128,S] f32 (12KB/part) + tp: 2 x [128,S] bf16
            # (4KB) = 16KB PSUM. Each tp tile owns a full 2KB zero region so
            # the V transposes don't invalidate each other.
            with (
                tc.tile_pool(name="pj", bufs=3, space="PSUM") as pj,
                tc.tile_pool(name="tp", bufs=2, space="PSUM") as tpp,
            ):
                def proj(w_ap_fn, tag):
                    ps = pj.tile([128, S], F32, tag="pj", name=f"ps_{tag}")
                    for ns in range(2):
                        for k in range(HC):
                            nc.tensor.matmul(
                                ps[:, ns * 512:(ns + 1) * 512],
                                w_ap_fn(k),
                                hsT[k][:, ns * 512: ns * 512 + 512],
                                start=(k == 0), stop=(k == HC - 1),
                            )
                    return ps

                def proj2(wa_fn, wb_fn, tag):
                    """Two projections interleaved per hsT chunk so the PE
                    keeps pace with the chunk-wise hsT DMA arrival."""
                    pa = pj.tile([128, S], F32, tag="pj", name=f"ps_{tag}a")
                    pb = pj.tile([128, S], F32, tag="pj", name=f"ps_{tag}b")
                    for k in range(HC):
                        for ps, w_fn in ((pa, wa_fn), (pb, wb_fn)):
                            for ns in range(2):
                                nc.tensor.matmul(
                                    ps[:, ns * 512:(ns + 1) * 512],
                                    w_fn(k),
                                    hsT[k][:, ns * 512: ns * 512 + 512],
                                    start=(k == 0), stop=(k == HC - 1),
                                )
                    return pa, pb

                def rot_cast(ps, tag):
                    # ACT cast to bf16 per 512-col half: subtile deps let the
                    # first half start while the projection's ns=1 matmuls
                    # still run, and ACT is idle pre-attention anyway
                    raw = wp.tile([128, S], BF16, tag="raw", name=f"raw_{tag}")
                    for ns in range(2):
                        sl = slice(ns * 512, (ns + 1) * 512)
                        nc.scalar.activation(
                            raw[:, sl], ps[:, sl],
                            mybir.ActivationFunctionType.Copy
                        )
                    return raw

                def rot_apply(raw, dsts, tag, pool=None, pool_tag="pj"):
                    """PE rotate-half matmul + DVE combine with cos/sin, per
                    512-col half to shorten the dependency chain. dsts:
                    (dst_tile, row_lo, row_hi, col_base)."""
                    pl = pj if pool is None else pool
                    ps_rot = pl.tile([128, S], F32, tag=pool_tag,
                                     name=f"rot_{tag}")
                    t1 = wp.tile([128, S], BF16, tag="t1", name=f"t1_{tag}")
                    t2 = wp.tile([128, S], BF16, tag="t2", name=f"t2_{tag}")
                    for ns in range(2):
                        sl = slice(ns * 512, (ns + 1) * 512)
                        nc.tensor.matmul(
                            ps_rot[:, sl], permT[:], raw[:, sl],
                            start=True, stop=True,
                        )
                        nc.vector.tensor_tensor(
                            t1[:, sl], raw[:, sl], cos2[:, sl],
                            mybir.AluOpType.mult
                        )
                        nc.vector.tensor_tensor(
                            t2[:, sl], ps_rot[:, sl], sin2[:, sl],
                            mybir.AluOpType.mult
                        )
                        for dst, lo, hi, col0 in dsts:
                            nc.vector.tensor_tensor(
                                dst[lo:hi, col0 + ns * 512:col0 + ns * 512
                                    + 512],
                                t1[lo:hi, sl], t2[lo:hi, sl],
                                mybir.AluOpType.add,
                            )

                def wq_ap(mc):
                    return lambda k: wqb[:, mc * HC * 128 + k * 128:
                                         mc * HC * 128 + (k + 1) * 128]

                def qdst(mc):
                    return [(qrot, 0, 128, mc * S)]

                # K and Q0 projections interleaved per hsT chunk (chase the
                # chunk-wise hsT DMA arrival); V proj next covers the rot
                # cast/combine latency of K and Q0; each later Q rot lags one
                # projection so its chain hides under the next proj's MMs
                ps_q = [None] * QC
                raw_q = [None] * QC
                ps_k, ps_q[0] = proj2(
                    lambda k: wk[:, k * 128:(k + 1) * 128],
                    wq_ap(0),
                    "kq0",
                )
                raw_k = rot_cast(ps_k, "k")
                raw_q[0] = rot_cast(ps_q[0], "q0")
                # V proj as V^T (stationary wv)
                ps_vt = proj(lambda k: wv[:, k * 128:(k + 1) * 128], "v")
                rot_apply(raw_k, [
                    (krepE, 0, 64, 0),
                    (krepO, 64, 128, 0),
                ], "k")
                rot_apply(raw_q[0], qdst(0), "q0")
                vt_sb = wp.tile([128, S], BF16, tag="vts")
                nc.scalar.activation(
                    vt_sb[:], ps_vt[:], mybir.ActivationFunctionType.Copy
                )
                ps_q[1] = proj(wq_ap(1), "q1")
                raw_q[1] = rot_cast(ps_q[1], "q1")
                # PE-transpose each token chunk into vaugE/vaugO; each
                # transpose gets its own PSUM bank (2-tile rotation)
                for t in range(KC):
                    ps_tr = tpp.tile([128, S], BF16, tag="tp", name=f"tr{t}")
                    nc.tensor.transpose(
                        ps_tr[:, 0:128], vt_sb[:, t * 128:(t + 1) * 128],
                        ident[:]
                    )
                    nc.vector.tensor_copy(
                        vaugE[:, t * 128:t * 128 + 64], ps_tr[:, 0:64]
                    )
                    nc.vector.tensor_copy(
                        vaugO[:, t * 128 + 64:(t + 1) * 128], ps_tr[:, 64:128]
                    )
                for mc in range(2, QC):
                    ps_q[mc] = proj(wq_ap(mc), f"q{mc}")
                    raw_q[mc] = rot_cast(ps_q[mc], f"q{mc}")
                    if mc < QC - 1:
                        rot_apply(raw_q[mc - 1], qdst(mc - 1), f"q{mc-1}")
                # rots of the last two Q pairs are emitted inside the
                # attention block (ps_rot from the st pool) so their DVE
                # chains hide under the first head's scores/exp stream —
                # qrot chunks 2/3 are only needed from head 4 onward

            # ================= attention =================
            # PSUM: st 2 x [128,S] f32 (8KB) + av 2 x [128,S] f32 (8KB) = 16KB
            def normalize(h, dsrc, psrc, staged=False):
                """Denominator row dr of dsrc -> reciprocal -> broadcast ->
                scale psrc's PV rows into attnT. staged=True when dsrc/psrc
                already live in SBUF."""
                mc, par = h // 2, h % 2
                r = par * 64
                dr = 64 if par == 0 else 0
                recip = wp.tile([128, S], F32, tag="recip", name=f"recip{h}")
                # ~18-bit approx reciprocal: single DVE op (the exact
                # `reciprocal` is 8 cyc/elem on ONE lane here -> 6.5us).
                # The custom op only works at base partition 0 (PSUM reads
                # are fine), so even heads hop the denominator row to
                # partition 0 FIRST (DVE stage to SBUF since DMA can't read
                # PSUM, then DMA hop), then recip at partition 0.
                if dr != 0:
                    if not staged:     # denominator still in PSUM
                        den = wp.tile([128, S], F32, tag="den",
                                      name=f"den{h}")
                        nc.vector.tensor_copy(
                            den[dr:dr + 1, :], dsrc[dr:dr + 1, :]
                        )
                        hop_src = den
                    else:
                        hop_src = dsrc
                    den0 = wp.tile([1, S], F32, tag="den0", name=f"den0{h}")
                    nc.sync.dma_start(
                        out=den0[0:1, :], in_=hop_src[dr:dr + 1, :]
                    )
                    din = den0[0:1, :]
                else:
                    din = dsrc[0:1, :]
                nc.vector.reciprocal_approx_fast(recip[0:1, :], din)
                bc = wp.tile([128, S], F32, tag="bcast", name=f"bc{h}")
                nc.gpsimd.partition_broadcast(bc[:], recip[0:1, :])
                nc.vector.tensor_tensor(
                    attnT[r:r + 64, mc * S:(mc + 1) * S],
                    psrc[r:r + 64, :], bc[r:r + 64, :],
                    mybir.AluOpType.mult,
                )

            with (
                tc.tile_pool(name="st", bufs=2, space="PSUM") as stp,
                tc.tile_pool(name="av", bufs=2, space="PSUM") as avp,
            ):
                # last two Q rots draw their PSUM from the av pool: av_h0's
                # first PV comes ~4us into attention, so the rot chains are
                # fully hidden and never stall the scores/exp rotation
                rot_apply(raw_q[QC - 2], qdst(QC - 2), f"q{QC-2}",
                          pool=avp, pool_tag="av")
                rot_apply(raw_q[QC - 1], qdst(QC - 1), f"q{QC-1}",
                          pool=avp, pool_tag="av")
                for h in range(QH):
                    mc = h // 2                  # q chunk / pair
                    par = h % 2                  # kv head = parity
                    r = par * 64                 # partition row base
                    krep = krepE if par == 0 else krepO

                    def scores(kc):
                        ps_st = stp.tile([128, S], F32, tag="st",
                                         name=f"st_h{h}k{kc}")
                        for ns in range(2):
                            nc.tensor.matmul(
                                ps_st[:, ns * 512:(ns + 1) * 512],
                                krep[:, kc * 128:(kc + 1) * 128],
                                qrot[:,
                                     mc * S + ns * 512: mc * S + ns * 512 + 512],
                                start=True, stop=True,
                            )
                        return ps_st

                    ps_at = avp.tile([128, S], F32, tag="av", name=f"av_h{h}")
                    st_tiles = {0: scores(0)}
                    for kc in range(KC):
                        if kc + 1 < KC:
                            st_tiles[kc + 1] = scores(kc + 1)
                        ps_st = st_tiles.pop(kc)
                        if use_mask:
                            nc.vector.tensor_tensor(
                                ps_st[:], ps_st[:],
                                maskT[:, kc * S:(kc + 1) * S],
                                mybir.AluOpType.add,
                            )
                        ex = ep.tile([128, S], BF16, tag="ex",
                                     name=f"ex_h{h}k{kc}")
                        nc.scalar.activation(
                            ex[:], ps_st[:], mybir.ActivationFunctionType.Exp
                        )
                        va = vaugE if par == 0 else vaugO
                        for ns in range(2):
                            nc.tensor.matmul(
                                ps_at[:, ns * 512:(ns + 1) * 512],
                                va[:, kc * 128:(kc + 1) * 128],
                                ex[:, ns * 512:(ns + 1) * 512],
                                start=(kc == 0), stop=(kc == KC - 1),
                            )
                    # normalize; denominator on row 64 (even) / row 0 (odd);
                    # even heads hop the reciprocal row to partition 0 for
                    # the gpsimd broadcast (HW broadcast reads partition 0)
                    if h >= QH - 2:
                        # last pair: ONE full-tile copy stages PV rows AND
                        # the denominator row to SBUF so the PSUM pools can
                        # close right after the final PV
                        nc.vector.tensor_copy(lastU[h][:], ps_at[:])
                        if h == QH - 2:
                            # head 6's chain runs inline, from SBUF
                            normalize(h, lastU[h], lastU[h], staged=True)
                        continue
                    normalize(h, ps_at, ps_at)

            # ================= output projection (transposed out) ==========
            # head 7's deferred normalize runs here, overlapping the early
            # contraction steps (kc2<3 don't touch attnT chunk 3)
            with tc.tile_pool(name="wop", bufs=4, space="PSUM") as wop:
                normalize(QH - 1, lastU[QH - 1], lastU[QH - 1], staged=True)
                def op_mm(pso, mc2, ns, kc2):
                    nc.tensor.matmul(
                        pso[:, ns * 512:(ns + 1) * 512],
                        wo[:, kc2 * HID + mc2 * 128:
                           kc2 * HID + (mc2 + 1) * 128],
                        attnT[:, kc2 * S + ns * 512:
                              kc2 * S + ns * 512 + 512],
                        start=(kc2 == 0), stop=(kc2 == QC - 1),
                    )

                def op_fin(pso, mc2):
                    # bf16 partials: host sums the 4 row-parallel partials in
                    # f32; halves the output DMA (8MB -> 4MB per core)
                    osb = op_.tile([128, S], BF16, tag="osb")
                    # alternate PSUM->SBUF copies between DVE and ACT (both
                    # ~1.1us per [128,1024] f32) and the output-DMA issue
                    # between the two HWDGE queues so neither serializes
                    if mc2 % 2 == 0:
                        nc.vector.tensor_copy(osb[:], pso[:])
                        nc.sync.dma_start(
                            out=out_d[mc2 * 128:(mc2 + 1) * 128, :], in_=osb[:]
                        )
                    else:
                        nc.scalar.activation(
                            osb[:], pso[:], mybir.ActivationFunctionType.Copy
                        )
                        nc.scalar.dma_start(
                            out=out_d[mc2 * 128:(mc2 + 1) * 128, :], in_=osb[:]
                        )

                # first 4 output chunks pre-accumulate kc2=0..2 while head
                # 7's deferred normalize chain runs (kc2<3 don't need it)
                NPRE = 4
                pre = []
                for mc2 in range(NPRE):
                    pso = wop.tile([128, S], F32, tag="wop")
                    pre.append(pso)
                    for ns in range(2):
                        for kc2 in range(QC - 1):
                            op_mm(pso, mc2, ns, kc2)
                for mc2 in range(NPRE):
                    pso = pre[mc2]
                    for ns in range(2):
                        op_mm(pso, mc2, ns, QC - 1)
                    op_fin(pso, mc2)
                for mc2 in range(NPRE, HID // 128):
                    pso = wop.tile([128, S], F32, tag="wop")
                    for ns in range(2):
                        for kc2 in range(QC):
                            op_mm(pso, mc2, ns, kc2)
                    op_fin(pso, mc2)
    nc.finalize()
    return nc


def _rope_tables():
    inv = 1.0 / (ROPE_BASE ** (np.arange(0, HD, 2, dtype=np.float32) / HD))
    t = np.arange(S, dtype=np.float32)
    freqs = np.outer(t, inv)
    emb = np.concatenate([freqs, freqs], axis=-1)  # [S, HD]
    return np.cos(emb).astype(np.float32), np.sin(emb).astype(np.float32)


def _perm_T():
    P = np.zeros((128, 128), dtype=np.float32)
    for blk in range(2):
        o = blk * 64
        for i in range(32):
            P[o + i, o + i + 32] = -1.0
            P[o + i + 32, o + i] = 1.0
    return P.T.astype(ml_dtypes.bfloat16)


# local head order: pair mc = (kv0 head mc, kv1 head mc)
_HEAD_PERM = [0, 4, 1, 5, 2, 6, 3, 7]


def _head_cols(g):
    cols = []
    for lh in _HEAD_PERM:
        s0 = (g * QH + lh) * HD
        cols.append(np.arange(s0, s0 + HD))
    return np.concatenate(cols)


def _core_weights(g, Wq, Wk, Wv, Wo, scale):
    """Pack a core's weight slices into the SBUF tile layouts
    (partition-major, hid-chunked) so each DMA is contiguous."""
    bf = ml_dtypes.bfloat16
    cols = _head_cols(g)
    wq_c = (Wq[:, cols] * scale).astype(bf)          # [2048, 512]
    wq_pack = np.ascontiguousarray(                  # mc-major
        wq_c.reshape(HC, 128, QC, 128).transpose(1, 2, 0, 3)
        .reshape(128, QC * HC * 128))
    wk_c = Wk[:, g * KVH * HD:(g + 1) * KVH * HD].astype(bf)
    wk_pack = np.ascontiguousarray(
        wk_c.reshape(HC, 128, KVH * HD).transpose(1, 0, 2)
        .reshape(128, HC * KVH * HD))
    wv_c = Wv[:, g * KVH * HD:(g + 1) * KVH * HD].astype(bf)
    wv_pack = np.ascontiguousarray(
        wv_c.reshape(HC, 128, KVH * HD).transpose(1, 0, 2)
        .reshape(128, HC * KVH * HD))
    wo_c = Wo[cols, :].astype(bf)                    # [512, 2048]
    wo_pack = np.ascontiguousarray(
        wo_c.reshape(QC, 128, HID).transpose(1, 0, 2)
        .reshape(128, QC * HID))
    return {"wq": wq_pack, "wk": wk_pack, "wv": wv_pack, "wo": wo_pack}


def kernel(hidden_states, position_ids, attention_mask, Wq, Wk, Wv, Wo,
           _trace=False):
    global LAST_RESULT
    bf = ml_dtypes.bfloat16
    hidden_states = np.asarray(hidden_states, dtype=np.float32)
    Wq = np.asarray(Wq, dtype=np.float32)
    Wk = np.asarray(Wk, dtype=np.float32)
    Wv = np.asarray(Wv, dtype=np.float32)
    Wo = np.asarray(Wo, dtype=np.float32)
    mask = np.asarray(attention_mask, dtype=np.float32)
    pos = np.asarray(position_ids).astype(np.int64)

    use_mask = bool(np.any(mask))
    key = use_mask
    if key not in _CACHE:
        _CACHE[key] = _build(use_mask)
    nc = _CACHE[key]

    cos_t, sin_t = _rope_tables()
    permT = _perm_T()
    scale = 1.0 / np.sqrt(HD)

    hsT_b = [np.ascontiguousarray(hidden_states[b].T).astype(bf)
             for b in range(B)]
    cos2_b, sin2_b = [], []
    for b in range(B):
        cos2_b.append(np.ascontiguousarray(
            np.tile(cos_t[pos[b]].T, (2, 1))).astype(bf))
        sin2_b.append(np.ascontiguousarray(
            np.tile(sin_t[pos[b]].T, (2, 1))).astype(bf))
    if use_mask:
        maskT_full = np.ascontiguousarray(
            np.maximum(mask[:, 0], NEG_BIG).transpose(0, 2, 1)).astype(bf)

    in_maps = []
    for c in range(8):
        b, g = c // G, c % G
        m = _core_weights(g, Wq, Wk, Wv, Wo, scale)
        m.update({
            "hsT": hsT_b[b],
            "permT": permT,
            "cos2": cos2_b[b],
            "sin2": sin2_b[b],
        })
        if use_mask:
            m["maskT"] = maskT_full[b]
        in_maps.append(m)

    res = run_bass_kernel_spmd(nc, in_maps, core_ids=list(range(8)),
                               trace=_trace)
    LAST_RESULT = res
    out = np.zeros((B, S, HID), dtype=np.float32)
    for c in range(8):
        out[c // G] += res.results[c]["out"].astype(np.float32).T
    return out



# revision 44
# speedup vs baseline: 1.0352x; 1.0046x over previous
"""GQA attention (B=2,S=1024,HID=2048,NH=32,NKV=8,HD=64) on 8 TRN2 cores.

Sharding: core c -> batch b=c//4, head-group g=c%4 (8 q heads / 2 kv heads).
Core computes partial out[b] = attn(heads of g) @ Wo[rows of g]; host sums the
4 row-parallel partials per batch.

Head pairing: local q heads are reordered [0,4,1,5,2,6,3,7] so q chunk mc
holds (kv0 head mc) on partitions 0:64 and (kv1 head mc) on 64:128. K proj
then emits both kv heads in ONE [128,S] chunk (kv0 rows 0:64, kv1 rows
64:128) with no replication, and scores use krep[r:r+64] with r=64*(h%2).

Device dataflow (matmuls bf16 -> fp32 PSUM), software-pipelined so the PE
never sits on an ACT/DVE result (PE p-state ramps only under continuous
execution):
  K proj -> V proj (covers ACT copy of K) -> rot(K) -> Q0 -> V transposes
  -> Q1 -> rot(Q0) -> Q2 -> rot(Q1) -> Q3 -> rot(Q2) -> rot(Q3)
  -> per head h: sc(0); for kc: [sc(kc+1); exp(kc) on ACT; PV(kc)]
     even heads get the softmax denominator fused via augmented [V|1] lhsT
     (PSUM row 64); odd heads use a ones-column matmul into row 0
  -> normalize: DVE reciprocal -> gpsimd partition_broadcast -> DVE mult
  -> out proj (stationary Wo) -> DMA out [2048,1024] f32 partials
"""

import numpy as np
import ml_dtypes

import concourse.bass as bass
import concourse.bacc as bacc
import concourse.mybir as mybir
from concourse.tile import TileContext
from concourse.bass_utils import run_bass_kernel_spmd
from concourse.masks import make_identity

B, S, HID = 2, 1024, 2048
NH, NKV, HD = 32, 8, 64
G = 4                      # head groups (tensor-parallel degree per batch)
QH = NH // G               # 8 q heads per core
KVH = NKV // G             # 2 kv heads per core
QD = QH * HD               # 512
ROPE_BASE = 10000.0
BF16 = mybir.dt.bfloat16
F32 = mybir.dt.float32
NEG_BIG = float(np.finfo(np.float32).min)

KC = S // 128              # 8 k-token chunks
HC = HID // 128            # 16 hidden chunks
QC = QD // 128             # 4 q-dim chunks (head pairs)

LAST_RESULT = None
_CACHE = {}


def _build(use_mask: bool) -> bass.Bass:
    nc = bacc.Bacc(None, target_bir_lowering=False)
    # weights arrive pre-packed on host into the SBUF tile layout so every
    # DMA is a contiguous 128-partition copy (few descriptors, fast issue)
    hsT_d = nc.dram_tensor("hsT", [HID, S], BF16, kind="ExternalInput")
    wq_d = nc.dram_tensor("wq", [128, QC * HC * 128], BF16,
                          kind="ExternalInput")
    wk_d = nc.dram_tensor("wk", [128, HC * KVH * HD], BF16,
                          kind="ExternalInput")
    wv_d = nc.dram_tensor("wv", [128, HC * KVH * HD], BF16,
                          kind="ExternalInput")
    wo_d = nc.dram_tensor("wo", [128, QC * HID], BF16, kind="ExternalInput")
    cos_d = nc.dram_tensor("cos2", [128, S], BF16, kind="ExternalInput")
    sin_d = nc.dram_tensor("sin2", [128, S], BF16, kind="ExternalInput")
    perm_d = nc.dram_tensor("permT", [128, 128], BF16, kind="ExternalInput")
    if use_mask:
        mask_d = nc.dram_tensor("maskT", [S, S], BF16, kind="ExternalInput")
    out_d = nc.dram_tensor("out", [HID, S], BF16, kind="ExternalOutput")

    with TileContext(nc) as tc:
        with (
            tc.tile_pool(name="resid", bufs=1) as rp,
            tc.tile_pool(name="work", bufs=2) as wp,
            tc.tile_pool(name="exps", bufs=8) as ep,
            tc.tile_pool(name="outs", bufs=3) as op_,
        ):
            # ---- input DMAs: split across the two HWDGE queues (sync +
            # scalar) so descriptor issue (~0.6us each) doesn't serialize.
            # Order: K/V weights first, then hsT chunks (K/V proj chase
            # them), then wq block 0, rope tables, wq 1-3, wo last ----
            # Arrival order matches consumption order: each queue leads with
            # the weight its first projection needs (wk / wq block 0), then
            # its half of the hsT chunks (K+Q0 proj chase them chunk-wise),
            # then later-needed tensors.
            # wk/wq0 split into a 4-chunk head + tail: the first projection
            # matmuls gate on a 128KB transfer instead of 512KB (early DMA
            # bandwidth is contended across all 8 cores)
            wk = rp.tile([128, HC * KVH * HD], BF16, tag="wk")
            nc.sync.dma_start(out=wk[:, 0:512], in_=wk_d[:, 0:512])
            wqb = rp.tile([128, QC * HC * 128], BF16, tag="wqb")
            nc.scalar.dma_start(out=wqb[:, 0:512], in_=wq_d[:, 0:512])
            hsT = [None] * HC
            for k in range(HC):
                t = rp.tile([128, S], BF16, tag=f"hsT{k}", name=f"hsT{k}")
                eng = nc.sync if k % 2 == 0 else nc.scalar
                if k < 2:
                    # first chunks in halves: the opening matmuls gate on a
                    # 128KB transfer instead of 256KB
                    eng.dma_start(out=t[:, 0:512],
                                  in_=hsT_d[k * 128:(k + 1) * 128, 0:512])
                    eng.dma_start(out=t[:, 512:S],
                                  in_=hsT_d[k * 128:(k + 1) * 128, 512:S])
                else:
                    eng.dma_start(out=t[:],
                                  in_=hsT_d[k * 128:(k + 1) * 128, :])
                hsT[k] = t
                if k == 1:
                    nc.sync.dma_start(out=wk[:, 512:1024],
                                      in_=wk_d[:, 512:1024])
                    nc.scalar.dma_start(out=wqb[:, 512:1024],
                                        in_=wq_d[:, 512:1024])
                if k == 3:
                    nc.sync.dma_start(
                        out=wk[:, 1024:HC * KVH * HD],
                        in_=wk_d[:, 1024:HC * KVH * HD],
                    )
                    nc.scalar.dma_start(
                        out=wqb[:, 1024:HC * 128], in_=wq_d[:, 1024:HC * 128]
                    )
            permT = rp.tile([128, 128], BF16, tag="permT")
            nc.scalar.dma_start(out=permT[:], in_=perm_d[:, :])
            wv = rp.tile([128, HC * KVH * HD], BF16, tag="wv")
            nc.scalar.dma_start(out=wv[:], in_=wv_d[:, :])
            cos2 = rp.tile([128, S], BF16, tag="cos2")
            nc.scalar.dma_start(out=cos2[:], in_=cos_d[:, :])
            sin2 = rp.tile([128, S], BF16, tag="sin2")
            nc.scalar.dma_start(out=sin2[:], in_=sin_d[:, :])
            for mc in range(1, QC):
                nc.sync.dma_start(
                    out=wqb[:, mc * HC * 128:(mc + 1) * HC * 128],
                    in_=wq_d[:, mc * HC * 128:(mc + 1) * HC * 128],
                )
            wo = rp.tile([128, QC * HID], BF16, tag="wo")
            nc.sync.dma_start(out=wo[:], in_=wo_d[:, :])
            if use_mask:
                maskT = rp.tile([128, KC * S], BF16, tag="maskT")
                nc.sync.dma_start(
                    out=maskT[:].rearrange("p (k q) -> p k q", k=KC),
                    in_=mask_d[:, :].rearrange("(k p) q -> p k q", p=128),
                )
            # ---- persistent intermediates ----
            qrot = rp.tile([128, QC * S], BF16, tag="qrot")
            # krep variants zero-padded to full 128 contraction rows: even
            # heads use [K_kv0; 0], odd use [0; K_kv1] so every scores matmul
            # runs the full 128x128 PE array (zeros kill the other head's q).
            krepE = rp.tile([128, S], BF16, tag="krepE")
            nc.any.memset(krepE[64:128, :], 0.0)
            krepO = rp.tile([128, S], BF16, tag="krepO")
            nc.any.memset(krepO[0:64, :], 0.0)
            # PV lhsT tiles, full 128 cols per kc chunk so every PV matmul
            # runs the whole PE array with the softmax denominator fused:
            #   vaugE = [V_kv0 | ones]: PV rows 0:64, denominator on row 64+
            #   vaugO = [ones | V_kv1]: denominator on rows 0:63, PV 64:128
            vaugE = rp.tile([128, KC * 128], BF16, tag="vaugE")
            nc.any.memset(vaugE[:], 1.0)
            vaugO = rp.tile([128, KC * 128], BF16, tag="vaugO")
            nc.any.memset(vaugO[:], 1.0)
            attnT = rp.tile([128, QC * S], BF16, tag="attnT")
            ident = rp.tile([128, 128], BF16, tag="ident")
            make_identity(nc, ident[:])
            # SBUF staging for the last pair so the attention PSUM pools can
            # close right after the final PV (normalize finishes during the
            # out-projection instead of serializing before it)
            lastU = {6: rp.tile([128, S], F32, tag="lastU6", name="lastU6"),
                     7: rp.tile([128, S], F32, tag="lastU7", name="lastU7")}

            # ================= projections + rope =================
            # pj: 3 bufs x [128,S] f32 (12KB/part) + tp: 2 x [128,S] bf16
            # (4KB) = 16KB PSUM. Each tp tile owns a full 2KB zero region so
            # the V transposes don't invalidate each other.
            with (
                tc.tile_pool(name="pj", bufs=3, space="PSUM") as pj,
                tc.tile_pool(name="tp", bufs=2, space="PSUM") as tpp,
            ):
                def proj(w_ap_fn, tag):
                    ps = pj.tile([128, S], F32, tag="pj", name=f"ps_{tag}")
                    for ns in range(2):
                        for k in range(HC):
                            nc.tensor.matmul(
                                ps[:, ns * 512:(ns + 1) * 512],
                                w_ap_fn(k),
                                hsT[k][:, ns * 512: ns * 512 + 512],
                                start=(k == 0), stop=(k == HC - 1),
                            )
                    return ps

                def proj2(wa_fn, wb_fn, tag):
                    """Two projections interleaved per hsT chunk so the PE
                    keeps pace with the chunk-wise hsT DMA arrival."""
                    pa = pj.tile([128, S], F32, tag="pj", name=f"ps_{tag}a")
                    pb = pj.tile([128, S], F32, tag="pj", name=f"ps_{tag}b")
                    for k in range(HC):
                        for ps, w_fn in ((pa, wa_fn), (pb, wb_fn)):
                            for ns in range(2):
                                nc.tensor.matmul(
                                    ps[:, ns * 512:(ns + 1) * 512],
                                    w_fn(k),
                                    hsT[k][:, ns * 512: ns * 512 + 512],
                                    start=(k == 0), stop=(k == HC - 1),
                                )
                    return pa, pb

                def rot_cast(ps, tag):
                    # ACT cast to bf16 per 512-col half: subtile deps let the
                    # first half start while the projection's ns=1 matmuls
                    # still run, and ACT is idle pre-attention anyway
                    raw = wp.tile([128, S], BF16, tag="raw", name=f"raw_{tag}")
                    for ns in range(2):
                        sl = slice(ns * 512, (ns + 1) * 512)
                        nc.scalar.activation(
                            raw[:, sl], ps[:, sl],
                            mybir.ActivationFunctionType.Copy
                        )
                    return raw

                def rot_apply(raw, dsts, tag, pool=None, pool_tag="pj"):
                    """PE rotate-half matmul + DVE combine with cos/sin, per
                    512-col half to shorten the dependency chain. dsts:
                    (dst_tile, row_lo, row_hi, col_base)."""
                    pl = pj if pool is None else pool
                    ps_rot = pl.tile([128, S], F32, tag=pool_tag,
                                     name=f"rot_{tag}")
                    t1 = wp.tile([128, S], BF16, tag="t1", name=f"t1_{tag}")
                    t2 = wp.tile([128, S], BF16, tag="t2", name=f"t2_{tag}")
                    for ns in range(2):
                        sl = slice(ns * 512, (ns + 1) * 512)
                        nc.tensor.matmul(
                            ps_rot[:, sl], permT[:], raw[:, sl],
                            start=True, stop=True,
                        )
                        nc.vector.tensor_tensor(
                            t1[:, sl], raw[:, sl], cos2[:, sl],
                            mybir.AluOpType.mult
                        )
                        nc.vector.tensor_tensor(
                            t2[:, sl], ps_rot[:, sl], sin2[:, sl],
                            mybir.AluOpType.mult
                        )
                        for dst, lo, hi, col0 in dsts:
                            nc.vector.tensor_tensor(
                                dst[lo:hi, col0 + ns * 512:col0 + ns * 512
                                    + 512],
                                t1[lo:hi, sl], t2[lo:hi, sl],
                                mybir.AluOpType.add,
                            )

                def wq_ap(mc):
                    return lambda k: wqb[:, mc * HC * 128 + k * 128:
                                         mc * HC * 128 + (k + 1) * 128]

                def qdst(mc):
                    return [(qrot, 0, 128, mc * S)]

                # K and Q0 projections interleaved per hsT chunk (chase the
                # chunk-wise hsT DMA arrival); V proj next covers the rot
                # cast/combine latency of K and Q0; each later Q rot lags one
                # projection so its chain hides under the next proj's MMs
                ps_q = [None] * QC
                raw_q = [None] * QC
                ps_k, ps_q[0] = proj2(
                    lambda k: wk[:, k * 128:(k + 1) * 128],
                    wq_ap(0),
                    "kq0",
                )
                raw_k = rot_cast(ps_k, "k")
                raw_q[0] = rot_cast(ps_q[0], "q0")
                # V proj as V^T (stationary wv)
                ps_vt = proj(lambda k: wv[:, k * 128:(k + 1) * 128], "v")
                rot_apply(raw_k, [
                    (krepE, 0, 64, 0),
                    (krepO, 64, 128, 0),
                ], "k")
                rot_apply(raw_q[0], qdst(0), "q0")
                vt_sb = wp.tile([128, S], BF16, tag="vts")
                nc.scalar.activation(
                    vt_sb[:], ps_vt[:], mybir.ActivationFunctionType.Copy
                )
                ps_q[1] = proj(wq_ap(1), "q1")
                raw_q[1] = rot_cast(ps_q[1], "q1")
                # PE-transpose each token chunk into vaugE/vaugO; each
                # transpose gets its own PSUM bank (2-tile rotation)
                for t in range(KC):
                    ps_tr = tpp.tile([128, S], BF16, tag="tp", name=f"tr{t}")
                    nc.tensor.transpose(
                        ps_tr[:, 0:128], vt_sb[:, t * 128:(t + 1) * 128],
                        ident[:]
                    )
                    nc.vector.tensor_copy(
                        vaugE[:, t * 128:t * 128 + 64], ps_tr[:, 0:64]
                    )
                    nc.vector.tensor_copy(
                        vaugO[:, t * 128 + 64:(t + 1) * 128], ps_tr[:, 64:128]
                    )
                for mc in range(2, QC):
                    ps_q[mc] = proj(wq_ap(mc), f"q{mc}")
                    raw_q[mc] = rot_cast(ps_q[mc], f"q{mc}")
                    if mc < QC - 1:
                        rot_apply(raw_q[mc - 1], qdst(mc - 1), f"q{mc-1}")
                # rots of the last two Q pairs are emitted inside the
                # attention block (ps_rot from the st pool) so their DVE
                # chains hide under the first head's scores/exp stream —
                # qrot chunks 2/3 are only needed from head 4 onward

            # ================= attention =================
            # PSUM: st 2 x [128,S] f32 (8KB) + av 2 x [128,S] f32 (8KB) = 16KB
            def normalize(h, dsrc, psrc, staged=False):
                """Denominator row dr of dsrc -> reciprocal -> broadcast ->
                scale psrc's PV rows into attnT. staged=True when dsrc/psrc
                already live in SBUF."""
                mc, par = h // 2, h % 2
                r = par * 64
                dr = 64 if par == 0 else 0
                recip = wp.tile([128, S], F32, tag="recip", name=f"recip{h}")
                # ~18-bit approx reciprocal: single DVE op (the exact
                # `reciprocal` is 8 cyc/elem on ONE lane here -> 6.5us).
                # The custom op only works at base partition 0 (PSUM reads
                # are fine), so even heads hop the denominator row to
                # partition 0 FIRST (DVE stage to SBUF since DMA can't read
                # PSUM, then DMA hop), then recip at partition 0.
                if dr != 0:
                    if not staged:     # denominator still in PSUM
                        den = wp.tile([128, S], F32, tag="den",
                                      name=f"den{h}")
                        nc.vector.tensor_copy(
                            den[dr:dr + 1, :], dsrc[dr:dr + 1, :]
                        )
                        hop_src = den
                    else:
                        hop_src = dsrc
                    den0 = wp.tile([1, S], F32, tag="den0", name=f"den0{h}")
                    nc.sync.dma_start(
                        out=den0[0:1, :], in_=hop_src[dr:dr + 1, :]
                    )
                    din = den0[0:1, :]
                else:
                    din = dsrc[0:1, :]
                nc.vector.reciprocal_approx_fast(recip[0:1, :], din)
                bc = wp.tile([128, S], F32, tag="bcast", name=f"bc{h}")
                nc.gpsimd.partition_broadcast(bc[:], recip[0:1, :])
                nc.vector.tensor_tensor(
                    attnT[r:r + 64, mc * S:(mc + 1) * S],
                    psrc[r:r + 64, :], bc[r:r + 64, :],
                    mybir.AluOpType.mult,
                )

            with (
                tc.tile_pool(name="st", bufs=2, space="PSUM") as stp,
                tc.tile_pool(name="av", bufs=2, space="PSUM") as avp,
            ):
                # last two Q rots draw their PSUM from the av pool: av_h0's
                # first PV comes ~4us into attention, so the rot chains are
                # fully hidden and never stall the scores/exp rotation
                rot_apply(raw_q[QC - 2], qdst(QC - 2), f"q{QC-2}",
                          pool=avp, pool_tag="av")
                rot_apply(raw_q[QC - 1], qdst(QC - 1), f"q{QC-1}",
                          pool=avp, pool_tag="av")
                for h in range(QH):
                    mc = h // 2                  # q chunk / pair
                    par = h % 2                  # kv head = parity
                    r = par * 64                 # partition row base
                    krep = krepE if par == 0 else krepO

                    def scores(kc):
                        ps_st = stp.tile([128, S], F32, tag="st",
                                         name=f"st_h{h}k{kc}")
                        for ns in range(2):
                            nc.tensor.matmul(
                                ps_st[:, ns * 512:(ns + 1) * 512],
                                krep[:, kc * 128:(kc + 1) * 128],
                                qrot[:,
                                     mc * S + ns * 512: mc * S + ns * 512 + 512],
                                start=True, stop=True,
                            )
                        return ps_st

                    ps_at = avp.tile([128, S], F32, tag="av", name=f"av_h{h}")
                    st_tiles = {0: scores(0)}
                    for kc in range(KC):
                        if kc + 1 < KC:
                            st_tiles[kc + 1] = scores(kc + 1)
                        ps_st = st_tiles.pop(kc)
                        if use_mask:
                            nc.vector.tensor_tensor(
                                ps_st[:], ps_st[:],
                                maskT[:, kc * S:(kc + 1) * S],
                                mybir.AluOpType.add,
                            )
                        ex = ep.tile([128, S], BF16, tag="ex",
                                     name=f"ex_h{h}k{kc}")
                        nc.scalar.activation(
                            ex[:], ps_st[:], mybir.ActivationFunctionType.Exp
                        )
                        va = vaugE if par == 0 else vaugO
                        for ns in range(2):
                            nc.tensor.matmul(
                                ps_at[:, ns * 512:(ns + 1) * 512],
                                va[:, kc * 128:(kc + 1) * 128],
                                ex[:, ns * 512:(ns + 1) * 512],
                                start=(kc == 0), stop=(kc == KC - 1),
                            )
                    # normalize; denominator on row 64 (even) / row 0 (odd);
                    # even heads hop the reciprocal row to partition 0 for
                    # the gpsimd broadcast (HW broadcast reads partition 0)
                    if h >= QH - 2:
                        # last pair: ONE full-tile copy stages PV rows AND
                        # the denominator row to SBUF so the PSUM pools can
                        # close right after the final PV. Head 7's copy runs
                        # on ACT (idle once the last exp retired) so the
                        # bank handoff to the out-proj pool isn't serialized
                        # behind DVE.
                        if h == QH - 1:
                            nc.scalar.activation(
                                lastU[h][:], ps_at[:],
                                mybir.ActivationFunctionType.Copy
                            )
                        else:
                            nc.vector.tensor_copy(lastU[h][:], ps_at[:])
                            # head 6's chain runs inline, from SBUF
                            normalize(h, lastU[h], lastU[h], staged=True)
                        continue
                    normalize(h, ps_at, ps_at)

            # ================= output projection (transposed out) ==========
            # head 7's deferred normalize runs here, overlapping the early
            # contraction steps (kc2<3 don't touch attnT chunk 3)
            with tc.tile_pool(name="wop", bufs=4, space="PSUM") as wop:
                normalize(QH - 1, lastU[QH - 1], lastU[QH - 1], staged=True)
                def op_mm(pso, mc2, ns, kc2):
                    nc.tensor.matmul(
                        pso[:, ns * 512:(ns + 1) * 512],
                        wo[:, kc2 * HID + mc2 * 128:
                           kc2 * HID + (mc2 + 1) * 128],
                        attnT[:, kc2 * S + ns * 512:
                              kc2 * S + ns * 512 + 512],
                        start=(kc2 == 0), stop=(kc2 == QC - 1),
                    )

                def op_fin(pso, mc2):
                    # bf16 partials: host sums the 4 row-parallel partials in
                    # f32; halves the output DMA (8MB -> 4MB per core)
                    osb = op_.tile([128, S], BF16, tag="osb")
                    # alternate PSUM->SBUF copies between DVE and ACT (both
                    # ~1.1us per [128,1024] f32) and the output-DMA issue
                    # between the two HWDGE queues so neither serializes
                    if mc2 % 2 == 0:
                        nc.vector.tensor_copy(osb[:], pso[:])
                        nc.sync.dma_start(
                            out=out_d[mc2 * 128:(mc2 + 1) * 128, :], in_=osb[:]
                        )
                    else:
                        nc.scalar.activation(
                            osb[:], pso[:], mybir.ActivationFunctionType.Copy
                        )
                        nc.scalar.dma_start(
                            out=out_d[mc2 * 128:(mc2 + 1) * 128, :], in_=osb[:]
                        )

                # first 4 output chunks pre-accumulate kc2=0..2 while head
                # 7's deferred normalize chain runs (kc2<3 don't need it)
                NPRE = 4
                pre = []
                for mc2 in range(NPRE):
                    pso = wop.tile([128, S], F32, tag="wop")
                    pre.append(pso)
                    for ns in range(2):
                        for kc2 in range(QC - 1):
                            op_mm(pso, mc2, ns, kc2)
                for mc2 in range(NPRE):
                    pso = pre[mc2]
                    for ns in range(2):
                        op_mm(pso, mc2, ns, QC - 1)
                    op_fin(pso, mc2)
                for mc2 in range(NPRE, HID // 128):
                    pso = wop.tile([128, S], F32, tag="wop")
                    for ns in range(2):
                        for kc2 in range(QC):
                            op_mm(pso, mc2, ns, kc2)
                    op_fin(pso, mc2)
    nc.finalize()
    return nc


def _rope_tables():
    inv = 1.0 / (ROPE_BASE ** (np.arange(0, HD, 2, dtype=np.float32) / HD))
    t = np.arange(S, dtype=np.float32)
    freqs = np.outer(t, inv)
    emb = np.concatenate([freqs, freqs], axis=-1)  # [S, HD]
    return np.cos(emb).astype(np.float32), np.sin(emb).astype(np.float32)


def _perm_T():
    P = np.zeros((128, 128), dtype=np.float32)
    for blk in range(2):
        o = blk * 64
        for i in range(32):
            P[o + i, o + i + 32] = -1.0
            P[o + i + 32, o + i] = 1.0
    return P.T.astype(ml_dtypes.bfloat16)


# local head order: pair mc = (kv0 head mc, kv1 head mc)
_HEAD_PERM = [0, 4, 1, 5, 2, 6, 3, 7]


def _head_cols(g):
    cols = []
    for lh in _HEAD_PERM:
        s0 = (g * QH + lh) * HD
        cols.append(np.arange(s0, s0 + HD))
    return np.concatenate(cols)


def _core_weights(g, Wq, Wk, Wv, Wo, scale):
    """Pack a core's weight slices into the SBUF tile layouts
    (partition-major, hid-chunked) so each DMA is contiguous."""
    bf = ml_dtypes.bfloat16
    cols = _head_cols(g)
    wq_c = (Wq[:, cols] * scale).astype(bf)          # [2048, 512]
    wq_pack = np.ascontiguousarray(                  # mc-major
        wq_c.reshape(HC, 128, QC, 128).transpose(1, 2, 0, 3)
        .reshape(128, QC * HC * 128))
    wk_c = Wk[:, g * KVH * HD:(g + 1) * KVH * HD].astype(bf)
    wk_pack = np.ascontiguousarray(
        wk_c.reshape(HC, 128, KVH * HD).transpose(1, 0, 2)
        .reshape(128, HC * KVH * HD))
    wv_c = Wv[:, g * KVH * HD:(g + 1) * KVH * HD].astype(bf)
    wv_pack = np.ascontiguousarray(
        wv_c.reshape(HC, 128, KVH * HD).transpose(1, 0, 2)
        .reshape(128, HC * KVH * HD))
    wo_c = Wo[cols, :].astype(bf)                    # [512, 2048]
    wo_pack = np.ascontiguousarray(
        wo_c.reshape(QC, 128, HID).transpose(1, 0, 2)
        .reshape(128, QC * HID))
    return {"wq": wq_pack, "wk": wk_pack, "wv": wv_pack, "wo": wo_pack}


def kernel(hidden_states, position_ids, attention_mask, Wq, Wk, Wv, Wo,
           _trace=False):
    global LAST_RESULT
    bf = ml_dtypes.bfloat16
    hidden_states = np.asarray(hidden_states, dtype=np.float32)
    Wq = np.asarray(Wq, dtype=np.float32)
    Wk = np.asarray(Wk, dtype=np.float32)
    Wv = np.asarray(Wv, dtype=np.float32)
    Wo = np.asarray(Wo, dtype=np.float32)
    mask = np.asarray(attention_mask, dtype=np.float32)
    pos = np.asarray(position_ids).astype(np.int64)

    use_mask = bool(np.any(mask))
    key = use_mask
    if key not in _CACHE:
        _CACHE[key] = _build(use_mask)
    nc = _CACHE[key]

    cos_t, sin_t = _rope_tables()
    permT = _perm_T()
    scale = 1.0 / np.sqrt(HD)

    hsT_b = [np.ascontiguousarray(hidden_states[b].T).astype(bf)
             for b in range(B)]
    cos2_b, sin2_b = [], []
    for b in range(B):
        cos2_b.append(np.ascontiguousarray(
            np.tile(cos_t[pos[b]].T, (2, 1))).astype(bf))
        sin2_b.append(np.ascontiguousarray(
            np.tile(sin_t[pos[b]].T, (2, 1))).astype(bf))
    if use_mask:
        maskT_full = np.ascontiguousarray(
            np.maximum(mask[:, 0], NEG_BIG).transpose(0, 2, 1)).astype(bf)

    in_maps = []
    for c in range(8):
        b, g = c // G, c % G
        m = _core_weights(g, Wq, Wk, Wv, Wo, scale)
        m.update({
            "hsT": hsT_b[b],
            "permT": permT,
            "cos2": cos2_b[b],
            "sin2": sin2_b[b],
        })
        if use_mask:
            m["maskT"] = maskT_full[b]
        in_maps.append(m)

    res = run_bass_kernel_spmd(nc, in_maps, core_ids=list(range(8)),
                               trace=_trace)
    LAST_RESULT = res
    out = np.zeros((B, S, HID), dtype=np.float32)
    for c in range(8):
        out[c // G] += res.results[c]["out"].astype(np.float32).T
    return out

